# revision 17
# baseline (speedup 1.0000x reference)
"""Trainium2 Bass kernel for nn_DenoiseModule (diffraction removal + 2x2 Wiener).

Math reduction (validated against the jax reference):
  - The reference FFT2 acts on the (W, C) axes of the (B,H,W,C)-transposed
    image; the Gaussian mask factorizes a[h]*s[w] and is constant along the
    channel-frequency axis, so the C-axis FFT cancels exactly.
  - K = IFFT(s) truncated to +-12 taps. K's imaginary part is negligible
    (|Kim|/|Kre| ~ 0.9%) and all real taps are positive, so for x >= 0 the
    filtered magnitude is simply z = a[h] * (x (*) Kre) -- no abs/sqrt needed,
    making z LINEAR in x.
  - Linearity lets the Wiener local mean fold into conv weights:
    lM = 0.25*box2x2(z) = (0.25*(K (*) [1,1]_w)) (*) xh, where xh[h] =
    x'[h] + x'[h-1] is precomputed on host. Only box(z^2) needs a separate
    on-device square + banded-matmul box.
  - Wiener tail: out = z + rr*(lM - z), rr = 1/max(lvar/noise, 1),
    lvar = box(z^2)/4 - lM^2, noise = mean(lvar).

Device mapping (per core: 4 images x 3 channels = 12 channels, batch-parallel
across 8 cores). Per channel, data is W-on-partitions in 4 chunks of 128.
Input is loaded in a shifted layout (w offset -12, circular) so each conv
chunk needs one full 128-contract matmul plus one 24-contract edge matmul.
All matmul inputs/weights bf16 (fast weight load), PSUM f32, elementwise
split across DVE (custom fused ops), ACT, and GPSIMD.
"""
import numpy as np

B, C, H, W = 32, 3, 512, 512
NCORES = 8
BL = B // NCORES          # images per core
NCH = BL * C              # channels per core
P = 128
NW = W // P               # w-chunks
FD = NW * H               # flattened free dim per channel
TAP = 12
DR = 40.0
EDGE = 2 * TAP            # contract rows of the B (edge) conv block
# one-NR-step reciprocal constants (Chebyshev seed over the ~x*~x interval)
RC_A = -0.23549792
RC_B = 2.0017324
SPL = 1024                # out = zc+td: cols [0,SPL) on DVE, [SPL,FD) on GPSIMD


def _bf16(x):
    import ml_dtypes
    return np.asarray(x, np.float32).astype(ml_dtypes.bfloat16)


def _constants():
    x_lin = np.linspace(-256, 256, 512).astype(np.float64)
    g = np.exp(-(x_lin ** 2) / (2 * DR * DR))
    sh = (np.arange(512) + 256) % 512
    a = g[sh]                      # per-h scale (fft-order coords)
    K = np.fft.ifft(g[sh])
    d = np.minimum(np.arange(512), 512 - np.arange(512))
    K = np.where(d <= TAP, K, 0)
    taps = np.r_[K.real[-TAP:], K.real[:TAP + 1]].astype(np.float64)  # t=-12..12
    taps2 = (np.r_[taps, 0.0] + np.r_[0.0, taps]) * 0.25              # t=-12..13

    # conv lhsT blocks [contract, out]; A: full 128 rows, B: 24 edge rows.
    # xs row p of chunk j holds w_i = j*128 - 12 + p; edge row q holds
    # w_i = j*128 + 116 + q (from chunk j+1 of the shifted layout).
    pm = np.arange(P)
    t_a = pm[None, :] + 12 - pm[:, None]          # t = m + 12 - p
    q = np.arange(EDGE)
    t_b = pm[None, :] - 116 - q[:, None]          # t = m - 116 - q

    def band(tidx, tp, lo, hi):
        v = np.where((tidx >= lo) & (tidx <= hi), tp[np.clip(tidx - lo, 0, hi - lo)], 0.0)
        return v.astype(np.float32)

    wza = band(t_a, taps, -TAP, TAP)
    wzb = band(t_b, taps, -TAP, TAP)
    wma = band(t_a, taps2, -TAP, TAP + 1)
    wmb = band(t_b, taps2, -TAP, TAP + 1)
    # chunk-0 fix: reference's w-box zero-pads at w=0 (not circular), so
    # column m=0 must carry only the z[w=0] term: 0.25*(z[0,h]+z[0,h-1]).
    wma0 = wma.copy()
    wma0[:, 0] = 0.25 * band(t_a[:, 0:1], taps, -TAP, TAP)[:, 0]
    # box lhsT: bs[m] += 0.25*(sq[m] + sq[m-1]); edge: bs[0] += 0.25*sq_prev[127]
    bx = np.zeros((P, P), np.float32)
    np.fill_diagonal(bx, 0.25)
    bx[np.arange(P - 1), np.arange(1, P)] = 0.25
    bxe = np.zeros((P, P), np.float32)
    bxe[P - 1, 0] = 0.25
    return (a.astype(np.float32), _bf16(wza), _bf16(wzb), _bf16(wma),
            _bf16(wma0), _bf16(wmb), _bf16(bx), _bf16(bxe))


_PROG_CACHE = {}


def _install_custom_ops():
    """Register fused DVE ops:
    LVAR_ANT:      out = in1 - in0^2, accum_out = per-partition row-sum
    TD_WIENER_ANT: out = recip1(max(in0*s0, 1)) * in1  (one-NR reciprocal)
    """
    import concourse.dve_ops as dops
    from concourse.dve_spec import (Spec, Src0, Src1, C0, C1, C2, Zero, One,
                                    maxx, lower, _has_src1, AluOp, Bin)
    from concourse.dve_uop import DveOpSpec

    def reg(name, spec):
        for op in dops.OPS:
            if op.name == name:
                return op
        shas = {}
        for ver in ("v3", "v4"):
            tmp = DveOpSpec(name=name, opcode=17, uops=lower(spec, ver=ver),
                            rd1_en=_has_src1(spec))
            shas[ver] = tmp.sha(ver)
        op = dops.DveOp(name, spec, subdim=False, uops_sha=shas)
        dops.OPS.append(op)
        dops.CUSTOM_DVE_SPECS[op.name] = spec
        dops._SUB_OPCODE_FOR_NAME[op.name] = 1 + max(dops._SUB_OPCODE_FOR_NAME.values())
        return op

    def _ref_lvar(in0, in1, s0, s1, imm2):
        b = (in1.astype(np.float32) - in0.astype(np.float32) ** 2).astype(np.float32)
        return b, b.reshape(b.shape[0], -1).sum(axis=-1, keepdims=True)

    lvar_op = reg("LVAR_ANT", Spec(
        body=Src1 - Src0 * Src0, accum=AluOp.ADD, accum_init=Zero,
        reference=_ref_lvar))

    def _ref_td(in0, in1, s0, s1, imm2):
        xx = in0.astype(np.float32)
        nxx = (~xx.view(np.uint32)).view(np.float32)
        yy0 = (nxx * np.float32(s1)).astype(np.float32)
        yy1 = (yy0 * (np.float32(imm2) - xx * yy0)).astype(np.float32)
        return ((yy1 - np.float32(1.0)) * in1.astype(np.float32)).astype(np.float32)

    # td' = (recip1(dd) - 1) * diff with dd = max(lvar*invn, 1) from a
    # separate tensor_scalar; out = lM + td' (z never needs an SBUF copy:
    # z = lM - diff). 7 ALU stages -- the dd clamp wouldn't fit (9 > 8).
    nx = Bin(AluOp.BITWISE_NOT, Src0, Src0)
    y0 = nx * C1
    y1 = y0 * (C2 - Src0 * y0)
    td_op = reg("TD_WIENER_ANT", Spec(body=(y1 - One) * Src1, reference=_ref_td))
    return lvar_op, td_op, True


def _build_program():
    from contextlib import ExitStack
    import concourse.bacc as bacc
    import concourse.tile as tile
    from concourse import mybir

    f32 = mybir.dt.float32
    f32r = mybir.dt.float32r
    bf16 = mybir.dt.bfloat16
    Alu = mybir.AluOpType

    lvar_op, td_op, td_fused = _install_custom_ops()

    nc = bacc.Bacc(None)
    xs_in = nc.declare_dram_parameter("xs", [NCH, P, FD], bf16, isOutput=False)
    xsh_in = nc.declare_dram_parameter("xsh", [NCH, P, FD], bf16, isOutput=False)
    wza_in = nc.declare_dram_parameter("wza", [P, P], bf16, isOutput=False)
    wzb_in = nc.declare_dram_parameter("wzb", [EDGE, P], bf16, isOutput=False)
    wma_in = nc.declare_dram_parameter("wma", [P, P], bf16, isOutput=False)
    wma0_in = nc.declare_dram_parameter("wma0", [P, P], bf16, isOutput=False)
    wmb_in = nc.declare_dram_parameter("wmb", [EDGE, P], bf16, isOutput=False)
    bx_in = nc.declare_dram_parameter("bx", [P, P], bf16, isOutput=False)
    bxe_in = nc.declare_dram_parameter("bxe", [P, P], bf16, isOutput=False)
    ones_in = nc.declare_dram_parameter("ones", [P, 2], f32, isOutput=False)
    onesr_in = nc.declare_dram_parameter("onesr", [1, P], f32, isOutput=False)
    y_out = nc.declare_dram_parameter("y", [NCH, P, FD], bf16, isOutput=True)

    with tile.TileContext(nc) as tc, ExitStack() as ctx:
        cpool = ctx.enter_context(tc.tile_pool(name="consts", bufs=1))
        wtiles = {}
        for nm, src, rows in (("wza", wza_in, P), ("wzb", wzb_in, EDGE),
                              ("wma", wma_in, P), ("wma0", wma0_in, P),
                              ("wmb", wmb_in, EDGE), ("bx", bx_in, P),
                              ("bxe", bxe_in, P)):
            t = cpool.tile([rows, P], bf16, tag=nm)
            nc.sync.dma_start(t[:], src[:])
            wtiles[nm] = t
        ones_t = cpool.tile([P, 2], f32, tag="ones")
        nc.sync.dma_start(ones_t[:], ones_in[:])
        onesr_t = cpool.tile([1, P], f32, tag="onesr")
        nc.sync.dma_start(onesr_t[:], onesr_in[:])
        # persistent rotating sq tiles; 2 leading zero cols per chunk give the
        # h-1 zero-pad while keeping matmul reads aligned
        sq_tiles = []
        for k in range(3):
            t = cpool.tile([P, NW, H + 2], bf16, tag=f"sqt{k}")
            nc.vector.memset(t[:, :, 0:2], 0.0)
            sq_tiles.append(t)

        xpool = ctx.enter_context(tc.tile_pool(name="xin", bufs=4))
        wpool = ctx.enter_context(tc.tile_pool(name="work", bufs=4))
        tpool = ctx.enter_context(tc.tile_pool(name="tail", bufs=3))
        npool = ctx.enter_context(tc.tile_pool(name="noise", bufs=3))
        # z drains later than m (sq + diff consumers) -> give it 3 banks
        psz = ctx.enter_context(tc.tile_pool(name="psz", bufs=3, space="PSUM"))
        psm = ctx.enter_context(tc.tile_pool(name="psm", bufs=2, space="PSUM"))
        psb = ctx.enter_context(tc.tile_pool(name="psb", bufs=2, space="PSUM"))

        def emit_pass_a1(ch):
            """conv matmuls + psum drains (sq/lMc/diff). All z/m matmuls are
            emitted before any bs matmul so the in-order PE queue never stalls
            behind the ACT sq drains."""
            xin = xpool.tile([P, FD], bf16, tag="xs")
            nc.sync.dma_start(xin[:], xs_in[ch])
            xhin = xpool.tile([P, FD], bf16, tag="xsh")
            nc.sync.dma_start(xhin[:], xsh_in[ch])
            sq = sq_tiles[ch % 3]
            # f32-family everywhere the DVE reads: measured bf16 DVE ops run
            # at ~1.8ns/col (no 16-bit packing) vs 1.16 for f32
            lMc = wpool.tile([P, FD], f32r, tag="lMc")
            diff = wpool.tile([P, FD], f32, tag="diff")
            lvar = wpool.tile([P, FD], f32, tag="lvar")
            part = wpool.tile([P, NW], f32, tag="part")

            ps_zs, ps_ms = [], []
            for j in range(NW):
                c0, c1 = j * H, (j + 1) * H
                nj = (j + 1) % NW
                ps_z = psz.tile([P, H], f32, tag="z")
                nc.tensor.matmul(ps_z[:], wtiles["wza"][:], xin[:, c0:c1],
                                 start=True, stop=False)
                nc.tensor.matmul(ps_z[:], wtiles["wzb"][:],
                                 xin[0:EDGE, nj * H:(nj + 1) * H],
                                 start=False, stop=True)
                ps_m = psm.tile([P, H], f32, tag="m")
                nc.tensor.matmul(ps_m[:],
                                 wtiles["wma0" if j == 0 else "wma"][:],
                                 xhin[:, c0:c1], start=True, stop=False)
                nc.tensor.matmul(ps_m[:], wtiles["wmb"][:],
                                 xhin[0:EDGE, nj * H:(nj + 1) * H],
                                 start=False, stop=True)
                nc.scalar.square(sq[:, j, 2:H + 2], ps_z[:])
                nc.scalar.copy(lMc[:, c0:c1], ps_m[:])
                # diff = lM - z with one PSUM source: (z * -1) + lMc
                nc.vector.scalar_tensor_tensor(
                    diff[:, c0:c1], in0=ps_z[:], scalar=-1.0,
                    in1=lMc[:, c0:c1], op0=Alu.mult, op1=Alu.add)
                ps_zs.append(ps_z)
                ps_ms.append(ps_m)
            return {"sq": sq, "lMc": lMc, "diff": diff, "lvar": lvar,
                    "part": part, "ch": ch}

        def emit_pass_a2(st):
            """box matmuls on sq + lvar drain + noise scalar."""
            sq, lMc, lvar, part = st["sq"], st["lMc"], st["lvar"], st["part"]
            for j in range(NW):
                c0, c1 = j * H, (j + 1) * H
                ps_b = psb.tile([P, H], f32, tag="b")
                nc.tensor.matmul(ps_b[:], wtiles["bx"][:], sq[:, j, 2:H + 2],
                                 start=True, stop=False)
                nc.tensor.matmul(ps_b[:], wtiles["bx"][:], sq[:, j, 1:H + 1],
                                 start=False, stop=(j == 0))
                if j > 0:
                    nc.tensor.matmul(ps_b[:], wtiles["bxe"][:],
                                     sq[:, j - 1, 2:H + 2],
                                     start=False, stop=False)
                    nc.tensor.matmul(ps_b[:], wtiles["bxe"][:],
                                     sq[:, j - 1, 1:H + 1],
                                     start=False, stop=True)
                # DVE may read only ONE input from PSUM: lM from SBUF f32r
                nc.vector._custom_dve(lvar_op, out=lvar[:, c0:c1],
                                      in0=lMc[:, c0:c1], in1=ps_b[:],
                                      accum_out=part[:, j:j + 1])
            # noise scalar: PE reduce + broadcast (f32r tiles keep the PE out
            # of FP32-high mode, which would disable fast weight load)
            pr = npool.tile([P, 1], f32, tag="pr")
            nc.vector.tensor_reduce(pr[:], part[:], mybir.AxisListType.X, Alu.add)
            ps_n1 = psm.tile([P, H], f32, tag="m")
            nc.tensor.matmul(ps_n1[:1, :1], ones_t[:, 0:1], pr[:],
                             start=True, stop=True)
            nb = npool.tile([1, 1], f32, tag="nb")
            nc.scalar.copy(nb[:], ps_n1[:1, :1])
            ps_n2 = psb.tile([P, H], f32, tag="b")
            nc.tensor.matmul(ps_n2[:, :1], onesr_t[:], nb[:],
                             start=True, stop=True)
            noise = npool.tile([P, 1], f32, tag="noise")
            nc.scalar.mul(noise[:], ps_n2[:, :1], 1.0 / (H * W))
            invn = npool.tile([P, 1], f32, tag="invn")
            nc.vector.reciprocal_approx_fast(invn[:], noise[:])
            st["invn"] = invn
            return st

        def emit_pass_b(st):
            dd = tpool.tile([P, FD], f32, tag="dd")
            nc.vector.tensor_scalar(dd[:], st["lvar"][:], st["invn"][:], 1.0,
                                    Alu.mult, Alu.max)
            td = tpool.tile([P, FD], f32, tag="td")
            nc.vector._custom_dve(td_op, out=td[:], in0=dd[:],
                                  in1=st["diff"][:], s0=0.0,
                                  s1=RC_A, imm2=RC_B)
            out_t = tpool.tile([P, FD], bf16, tag="out")
            nc.gpsimd.tensor_tensor(out_t[:], st["lMc"][:], td[:], Alu.add)
            nc.scalar.dma_start(y_out[st["ch"]], out_t[:])

        # 3-stage software pipeline: while a2(ch-1)'s box matmuls wait on the
        # ACT sq drain, the PE has a1(ch)'s conv matmuls in front of them in
        # the queue -- no PE idle gap, so HAM stays at full clock.
        sts = {}
        for ch in range(NCH):
            sts[ch] = emit_pass_a1(ch)
            if ch >= 1:
                emit_pass_a2(sts[ch - 1])
            if ch >= 2:
                emit_pass_b(sts[ch - 2])
        emit_pass_a2(sts[NCH - 1])
        emit_pass_b(sts[NCH - 2])
        emit_pass_b(sts[NCH - 1])

    nc.finalize()
    return nc


def _get_prog():
    if "prog" not in _PROG_CACHE:
        (a, wza, wzb, wma, wma0, wmb, bx, bxe) = _constants()
        _PROG_CACHE.update(a=a, wza=wza, wzb=wzb, wma=wma, wma0=wma0,
                           wmb=wmb, bx=bx, bxe=bxe)
        _PROG_CACHE["prog"] = _build_program()
    return _PROG_CACHE["prog"]


def _prep_inputs(image):
    """Host: scale by a[h], build shifted-w layout + h-pair-sum, bf16."""
    import ml_dtypes
    a = _PROG_CACHE["a"]
    x = np.asarray(image, np.float32)
    xw = np.transpose(x, (0, 1, 3, 2)) * a[None, None, None, :]  # (B,C,W,H)
    xh = xw.copy()
    xh[..., 1:] += xw[..., :-1]
    widx = (np.arange(NW)[:, None] * P - TAP + np.arange(P)[None, :]) % W
    xs = xw[:, :, widx, :].transpose(0, 1, 3, 2, 4)   # (B,C,P,NW,H)
    xsh = xh[:, :, widx, :].transpose(0, 1, 3, 2, 4)
    xs = xs.reshape(NCORES, NCH, P, FD).astype(ml_dtypes.bfloat16)
    xsh = xsh.reshape(NCORES, NCH, P, FD).astype(ml_dtypes.bfloat16)
    return xs, xsh


def _run(image, **spmd_kwargs):
    from concourse.bass_utils import run_bass_kernel_spmd

    nc = _get_prog()
    xs, xsh = _prep_inputs(image)
    consts = {k: _PROG_CACHE[k] for k in ("wza", "wzb", "wma", "wma0",
                                          "wmb", "bx", "bxe")}
    consts["ones"] = np.ones((P, 2), np.float32)
    consts["onesr"] = np.ones((1, P), np.float32)
    in_maps = [{"xs": xs[c], "xsh": xsh[c], **consts} for c in range(NCORES)]
    res = run_bass_kernel_spmd(nc, in_maps, list(range(NCORES)), **spmd_kwargs)
    ys = np.stack([np.asarray(res.results[c]["y"]) for c in range(NCORES)])
    # y[core, ch, p, j*H+h] -> w = j*128 + p
    out = ys.astype(np.float32).reshape(B, C, P, NW, H)
    out = out.transpose(0, 1, 3, 2, 4).reshape(B, C, W, H)  # (B,C,w,h)
    out = np.ascontiguousarray(out.transpose(0, 1, 3, 2))
    return out, res


def kernel(image):
    out, _ = _run(image)
    return out
